# revision 17
# baseline (speedup 1.0000x reference)
"""CrossAttention (B=2, S=2048, D=1024, H=16, DH=64) on 8 TRN2 NeuronCores.

Megatron-style head sharding: core i owns heads {2i, 2i+1} (a 128-column
slice of Wq/Wk/Wv), computes attention for those heads over both batch
elements, all-gathers the per-head attention outputs across cores, then
computes a 128-column slice of the output projection.

v3 structure (the PE queue is strictly in-order, so emission order ==
tensor-engine execution order; everything is arranged so the PE never
waits long enough for the HAM clock-gate to drop it back to 1.2 GHz):

  phase 0: warmup matmuls on a memset tile while startup DMAs land
  phase 1: proj b0 (Q,K,V) + proj b1 (V only) + all V-transposes
  groups b0q0..b0q3: attention, weaving b1's Q (kb 0-7) and K (kb 8-15)
      projection matmuls one per kb-step
  groups b1q0..b1q3: attention, weaving outproj for group g-4 one
      matmul per kb-step (shares its PSUM bank with the b1-proj weave)
  tail: outproj for the last 4 groups

One small AllGather per (batch, q-chunk) group fires right after that
group's normalize; og gather reads are emitted two groups after their
collective so their semaphore wait never parks the sync queue.
Normalize: DVE approx-reciprocal straight from PSUM, gpsimd
partition_broadcast, DVE multiply.  Softmax skips max-subtraction
(scores bounded ~|2.5|) and fuses sum(exp) into attn@V via a
ones-column on V (65-row PSUM outputs).
"""
import os
import numpy as np
from contextlib import ExitStack

import bass_rust
from concourse import bacc
import concourse.bass as bass
import concourse.mybir as mybir
import concourse.tile as tile
from concourse.bass_utils import run_bass_kernel_spmd

F32R = mybir.dt.float32r
F32 = mybir.dt.float32
BF16 = mybir.dt.bfloat16

USE_BF16 = os.environ.get("KERNEL_BF16", "1") == "1"
MMDT = BF16 if USE_BF16 else F32R

B, S, D = 2, 2048, 1024
H, DH = 16, 64
NCORES = 8
T = B * S                 # 4096 tokens
HPC = H // NCORES         # 2 heads per core
W_SL = HPC * DH           # 128: per-core col-slice width of Wq/Wk/Wv and Wo
SCALE = DH ** -0.5        # 0.125
KB_D = D // 128           # 8 contraction blocks over D
QC = S // 512             # 4 query chunks per batch
KBS = S // 128            # 16 key blocks per batch
NG = B * QC               # 8 groups; order: g 0-3 = b0 qc 0-3, g 4-7 = b1
WEAVE_LAG = 5             # outproj for group g woven into group g+WEAVE_LAG
N_WARM = int(os.environ.get("KERNEL_WARM", "40"))

_NC_CACHE = {}


def build_nc():
    nc = bacc.Bacc(num_devices=NCORES)

    xt = nc.dram_tensor("xt", [D, T], MMDT, kind="ExternalInput")       # x^T
    wq = nc.dram_tensor("wq", [128, KB_D * W_SL], MMDT, kind="ExternalInput")  # pre-tiled
    wk = nc.dram_tensor("wk", [128, KB_D * W_SL], MMDT, kind="ExternalInput")
    wv = nc.dram_tensor("wv", [128, KB_D * W_SL], MMDT, kind="ExternalInput")
    wo = nc.dram_tensor("wo", [128, KB_D * W_SL], MMDT, kind="ExternalInput")
    bo = nc.dram_tensor("bo", [W_SL, 1], F32, kind="ExternalInput")     # bo col slice
    out = nc.dram_tensor("out", [W_SL, T], F32, kind="ExternalOutput")  # out^T slice

    o_loc = [nc.dram_tensor(f"o_loc{g}", [W_SL, 512], MMDT, kind="Internal")
             for g in range(NG)]
    o_gat = [nc.dram_tensor(f"o_gat{g}", [NCORES * W_SL, 512], MMDT,
                            kind="Internal", addr_space="Shared")
             for g in range(NG)]

    def g_bqc(g):
        return (0, g) if g < QC else (1, g - QC)

    xt_r = xt.ap().rearrange("(kb p) t -> p kb t", p=128)

    with tile.TileContext(nc) as tc, ExitStack() as ctx:
        wpool = ctx.enter_context(tc.tile_pool(name="wpool", bufs=1))
        xpool = ctx.enter_context(tc.tile_pool(name="xpool", bufs=3))
        x0pool = ctx.enter_context(tc.tile_pool(name="x0pool", bufs=1))
        xb1pool = ctx.enter_context(tc.tile_pool(name="xb1pool", bufs=1))
        proj = ctx.enter_context(tc.tile_pool(name="proj", bufs=1))
        epool = ctx.enter_context(tc.tile_pool(name="epool", bufs=4))
        npool = ctx.enter_context(tc.tile_pool(name="npool", bufs=2))
        outp = ctx.enter_context(tc.tile_pool(name="outp", bufs=2))
        ogpool = ctx.enter_context(tc.tile_pool(name="ogpool", bufs=3))

        # ---- startup DMAs: most-urgent first, spread across engine queues ----
        wq_sb = wpool.tile([128, KB_D, W_SL], MMDT, name="wq_sb")
        wk_sb = wpool.tile([128, KB_D, W_SL], MMDT, name="wk_sb")
        wv_sb = wpool.tile([128, KB_D, W_SL], MMDT, name="wv_sb")
        wo_sb = wpool.tile([128, KB_D, W_SL], MMDT, name="wo_sb")
        ident = wpool.tile([128, 128], MMDT, name="ident_sb")
        bo_sb = wpool.tile([W_SL, 1], F32, name="bo_sb")
        x0 = [x0pool.tile([128, 512], MMDT, name=f"x0_{kb}") for kb in range(KB_D)]
        xb1 = [xb1pool.tile([128, KB_D, 512], MMDT, name=f"xb1_{qc}")
               for qc in range(QC)]

        def w_ap(w_d):
            return w_d.ap().rearrange("p (kb m) -> p kb m", kb=KB_D)

        nc.sync.dma_start(out=wq_sb, in_=w_ap(wq))
        nc.scalar.dma_start(out=wk_sb, in_=w_ap(wk))
        dma_engs = (nc.sync, nc.scalar, nc.gpsimd)
        for kb in range(KB_D):
            dma_engs[kb % 3].dma_start(out=x0[kb], in_=xt_r[:, kb, 0:512])
        nc.gpsimd.dma_start(out=wv_sb, in_=w_ap(wv))
        xcs = {}
        for tcb in range(1, QC):
            gsl = slice(tcb * 512, (tcb + 1) * 512)
            xc = xpool.tile([128, KB_D, 512], MMDT, name="xc")
            nc.sync.dma_start(out=xc, in_=xt_r[:, :, gsl])
            xcs[tcb] = xc

        np_dt = np.float32 if MMDT is F32R else np.dtype("bfloat16")
        try:
            eye = np.eye(128, dtype=np_dt)
        except TypeError:
            import ml_dtypes
            eye = np.eye(128, dtype=ml_dtypes.bfloat16)
        ident_d = nc.inline_tensor(np.eye(128, dtype=np.float32) if MMDT is F32R
                                   else eye, name="ident")
        nc.scalar.dma_start(out=ident, in_=ident_d.ap().bitcast(MMDT))
        for qc in range(QC):
            gsl = slice(S + qc * 512, S + (qc + 1) * 512)
            dma_engs[qc % 3].dma_start(out=xb1[qc], in_=xt_r[:, :, gsl])
        nc.scalar.dma_start(out=bo_sb, in_=bo.ap())
        nc.gpsimd.dma_start(out=wo_sb, in_=w_ap(wo))

        # per-batch projection outputs
        qT = [proj.tile([128, S], MMDT, name=f"qT{b}") for b in range(B)]
        kT = [proj.tile([128, S], MMDT, name=f"kT{b}") for b in range(B)]
        vT = [proj.tile([128, S], MMDT, name=f"vT{b}") for b in range(B)]
        v_aug = [proj.tile([128, KBS, 130], MMDT, name=f"v_aug{b}") for b in range(B)]
        warm_sb = wpool.tile([128, 256], MMDT, name="warm_sb")
        nc.vector.memset(warm_sb.bitcast(F32) if MMDT is F32R else warm_sb, 0.25)

        # tiny dummy collective: absorbs CC engine init + first-rendezvous cost
        ccw_loc = nc.dram_tensor("ccw_loc", [1, 64], MMDT, kind="Internal")
        ccw_gat = nc.dram_tensor("ccw_gat", [NCORES, 64], MMDT,
                                 kind="Internal", addr_space="Shared")
        wdma = nc.sync.dma_start(out=ccw_loc.ap(), in_=warm_sb[0:1, 0:64])
        ccw = nc.gpsimd.collective_compute(
            "AllGather", mybir.AluOpType.bypass,
            replica_groups=[list(range(NCORES))],
            ins=[ccw_loc.ap()], outs=[ccw_gat.ap()],
        )
        bass_rust.add_dep_helper(ccw.ins, wdma.ins, sync=True, reason="ccw after dma")

        with tc.tile_pool(name="pps", bufs=2, space="PSUM") as pps, \
             tc.tile_pool(name="tps", bufs=2, space="PSUM") as tps, \
             tc.tile_pool(name="wmp", bufs=1, space="PSUM") as wmp:
            # ---- phase 0: HAM warmup on memset data while startup DMAs land
            warm_ps = wmp.tile([128, 256], F32, name="warm_ps")
            for _ in range(N_WARM):
                nc.tensor.matmul(warm_ps, warm_sb[:, 0:128], warm_sb,
                                 start=True, stop=True)

            # ---- phase 1: proj b0 (QKV) + proj b1 (V) + all transposes ----
            def proj_chunk(b, tcb, xc_kb, which):
                sl = slice(tcb * 512, (tcb + 1) * 512)
                for w_sb, dst, cp in which:
                    acc = pps.tile([128, 512], F32, name="acc")
                    for kb in range(KB_D):
                        nc.tensor.matmul(acc, w_sb[:, kb, :], xc_kb(kb),
                                         start=(kb == 0), stop=(kb == KB_D - 1))
                    if cp == "s":
                        nc.scalar.copy(out=dst[:, sl], in_=acc)
                    else:
                        nc.vector.tensor_copy(dst[:, sl], acc)

            def transposes(b):
                for kb in range(KBS):
                    tp = tps.tile([128, 128], MMDT, name="tp")
                    nc.tensor.transpose(tp, vT[b][:, kb * 128:(kb + 1) * 128], ident)
                    nc.vector.tensor_copy(v_aug[b][:, kb, 0:64], tp[:, 0:64])
                    nc.vector.tensor_copy(v_aug[b][:, kb, 65:129], tp[:, 64:128])
                    for seg in (v_aug[b][:, kb, 64:65], v_aug[b][:, kb, 129:130]):
                        nc.gpsimd.memset(seg.bitcast(F32) if MMDT is F32R else seg, 1.0)

            for tcb in range(QC):
                if tcb == 0:
                    xc_kb = lambda kb: x0[kb]
                else:
                    xc_kb = lambda kb, xc=xcs[tcb]: xc[:, kb, :]
                proj_chunk(0, tcb, xc_kb,
                           ((wq_sb, qT[0], "s"), (wk_sb, kT[0], "s"),
                            (wv_sb, vT[0], "v")))
            transposes(0)
            for tcb in range(QC):
                proj_chunk(1, tcb, lambda kb, t=tcb: xb1[t][:, kb, :],
                           ((wv_sb, vT[1], "v"),))
            transposes(1)

        # ---- attention + woven projections ----
        cc_insts = []
        og_tiles = {}
        finished_w = set()

        with tc.tile_pool(name="aps", bufs=2, space="PSUM") as aps, \
             tc.tile_pool(name="ops", bufs=1, space="PSUM") as ops, \
             tc.tile_pool(name="wps", bufs=1, space="PSUM") as wps:

            def emit_og_dma(wg):
                og = ogpool.tile([128, KB_D, 512], MMDT, name="og")
                og_r = o_gat[wg].ap().rearrange("(kb p) t -> p kb t", p=128)
                g = nc.sync.dma_start(out=og, in_=og_r)
                bass_rust.add_dep_helper(g.ins, cc_insts[wg].ins,
                                         sync=True, reason="og after cc")
                og_tiles[wg] = og

            wacc_box = [None]

            def emit_w_mm(wg, kb):
                if kb == 0:
                    wacc_box[0] = wps.tile([128, 512], F32, name="wacc")
                nc.tensor.matmul(wacc_box[0], wo_sb[:, kb, :], og_tiles[wg][:, kb, :],
                                 start=(kb == 0), stop=(kb == KB_D - 1))

            def emit_w_finish(wg):
                wb, wqc = g_bqc(wg)
                osb2 = outp.tile([128, 512], F32, name="osb2")
                nc.vector.tensor_scalar_add(osb2, wacc_box[0], bo_sb[:, 0:1])
                nc.sync.dma_start(
                    out=out.ap()[:, wb * S + wqc * 512:wb * S + (wqc + 1) * 512],
                    in_=osb2)
                finished_w.add(wg)

            pacc_box = [None]

            def emit_p_mm(qc1, kb):
                # woven b1 Q (kb 0-7) / K (kb 8-15) projection matmul
                w_sb, kbp = (wq_sb, kb) if kb < KB_D else (wk_sb, kb - KB_D)
                if kbp == 0:
                    pacc_box[0] = wps.tile([128, 512], F32, name="wacc")
                nc.tensor.matmul(pacc_box[0], w_sb[:, kbp, :], xb1[qc1][:, kbp, :],
                                 start=(kbp == 0), stop=(kbp == KB_D - 1))

            def emit_p_copy(qc1, kb):
                dst = qT[1] if kb < KB_D else kT[1]
                sl = slice(qc1 * 512, (qc1 + 1) * 512)
                nc.vector.tensor_copy(dst[:, sl], pacc_box[0])

            for gi in range(NG):
                b, qc = g_bqc(gi)
                qsl = slice(qc * 512, (qc + 1) * 512)
                po = [ops.tile([65, 512], F32, name=f"po{h}") for h in range(HPC)]
                ps_tiles = {}
                et_tiles = {}

                def emit_scores(kb, b=b, qsl=qsl, ps_tiles=ps_tiles):
                    ps_s = aps.tile([128, 1024], F32, name="ps_s")
                    ps_tiles[kb] = ps_s
                    ksl = slice(kb * 128, (kb + 1) * 128)
                    for h in range(HPC):
                        hsl = slice(h * 64, (h + 1) * 64)
                        nc.tensor.matmul(
                            ps_s[:, h * 512:(h + 1) * 512],
                            kT[b][hsl, ksl], qT[b][hsl, qsl],
                            start=True, stop=True,
                            tile_position=(h * 64, 0),
                        )

                def emit_exp(kb, ps_tiles=ps_tiles, et_tiles=et_tiles):
                    et = epool.tile([128, 1024], MMDT, name="et")
                    et_tiles[kb] = et
                    nc.scalar.activation(out=et, in_=ps_tiles.pop(kb),
                                         func=mybir.ActivationFunctionType.Exp,
                                         scale=SCALE)

                def emit_attnv(kb, b=b, po=po, et_tiles=et_tiles):
                    et = et_tiles.pop(kb)
                    for h in range(HPC):
                        nc.tensor.matmul(
                            po[h][0:65, :],
                            v_aug[b][:, kb, h * 65:(h + 1) * 65],
                            et[:, h * 512:(h + 1) * 512],
                            start=(kb == 0), stop=(kb == KBS - 1),
                        )

                emit_scores(0)
                emit_exp(0)
                emit_scores(1)
                emit_exp(1)
                for kb in range(KBS - 1):
                    if kb >= 1:
                        emit_scores(kb + 1)
                        emit_exp(kb + 1)
                    emit_attnv(kb)
                    if gi < QC:
                        emit_p_mm(qc, kb)          # b1 proj weave
                        if kb == KB_D - 1:
                            emit_p_copy(qc, kb)
                    elif gi >= WEAVE_LAG:
                        wg = gi - WEAVE_LAG
                        if kb < KB_D:
                            emit_w_mm(wg, kb)      # outproj weave
                        elif kb == KB_D:
                            emit_w_finish(wg)
                    if kb == 4 and gi >= 2:
                        emit_og_dma(gi - 2)        # CC(gi-2) already done
                emit_attnv(KBS - 1)
                if gi < QC:
                    emit_p_mm(qc, KBS - 1)
                    emit_p_copy(qc, KBS - 1)

                # ---- normalize: copy po out of PSUM first (frees the banks
                # for the next group's attnV), then normalize from SBUF ----
                pos, sums = [], []
                for h in range(HPC):
                    p = npool.tile([64, 512], F32, name="pos")
                    nc.vector.tensor_copy(p, po[h][0:64, :])
                    s = npool.tile([1, 512], F32, name="sums_sb")
                    nc.vector.tensor_copy(s, po[h][64:65, :])
                    pos.append(p)
                    sums.append(s)
                oloc_dmas = []
                for h in range(HPC):
                    rec = npool.tile([1, 512], F32, name="rec")
                    nc.vector.reciprocal_approx_fast(out=rec, in_=sums[h])
                    rec_bc = npool.tile([64, 512], F32, name="rec_bc")
                    nc.gpsimd.partition_broadcast(rec_bc, rec, channels=64)
                    osb = npool.tile([64, 512], MMDT, name="osb")
                    nc.vector.tensor_mul(osb, pos[h], rec_bc)
                    d = nc.sync.dma_start(
                        out=o_loc[gi].ap()[h * 64:(h + 1) * 64, :], in_=osb)
                    oloc_dmas.append(d.ins)
                cc = nc.gpsimd.collective_compute(
                    "AllGather", mybir.AluOpType.bypass,
                    replica_groups=[list(range(NCORES))],
                    ins=[o_loc[gi].ap()], outs=[o_gat[gi].ap()],
                    unique_tensors="Yes",
                )
                for dd in oloc_dmas:
                    bass_rust.add_dep_helper(cc.ins, dd, sync=True,
                                             reason="cc after o_loc")
                cc_insts.append(cc)

            # ---- tail: remaining og reads + output projections ----
            for wg in range(NG):
                if wg in finished_w:
                    continue
                if wg not in og_tiles:
                    emit_og_dma(wg)
                for kb in range(KB_D):
                    emit_w_mm(wg, kb)
                emit_w_finish(wg)

    nc.finalize()
    return nc


def _tile_w(w, np_dt):
    # [D, W_SL] -> [128, KB_D*W_SL] matching sbuf tile [128, kb, m]
    return np.ascontiguousarray(
        w.reshape(KB_D, 128, W_SL).transpose(1, 0, 2).reshape(128, KB_D * W_SL)
    ).astype(np_dt)


def kernel(x, Wq, Wk, Wv, Wo, bo):
    import ml_dtypes
    np_dt = np.float32 if not USE_BF16 else ml_dtypes.bfloat16
    x = np.asarray(x, dtype=np.float32)
    Wq = np.asarray(Wq, dtype=np.float32)
    Wk = np.asarray(Wk, dtype=np.float32)
    Wv = np.asarray(Wv, dtype=np.float32)
    Wo = np.asarray(Wo, dtype=np.float32)
    bo = np.asarray(bo, dtype=np.float32)

    if "nc" not in _NC_CACHE:
        _NC_CACHE["nc"] = build_nc()
    nc = _NC_CACHE["nc"]

    xt = np.ascontiguousarray(x.reshape(T, D).T).astype(np_dt)  # [D, T]
    in_maps = []
    for c in range(NCORES):
        csl = slice(c * W_SL, (c + 1) * W_SL)
        in_maps.append({
            "xt": xt,
            "wq": _tile_w(Wq[:, csl], np_dt),
            "wk": _tile_w(Wk[:, csl], np_dt),
            "wv": _tile_w(Wv[:, csl], np_dt),
            "wo": _tile_w(Wo[:, csl], np_dt),
            "bo": np.ascontiguousarray(bo[csl]).reshape(W_SL, 1),
        })
    res = run_bass_kernel_spmd(nc, in_maps, core_ids=list(range(NCORES)))
    LAST_RESULT["exec_time_ns"] = res.exec_time_ns
    LAST_RESULT["scope_times"] = res.per_core_scope_times
    LAST_RESULT["trace"] = res.instructions_and_trace[1] if res.instructions_and_trace else None
    out_t = np.concatenate([res.results[c]["out"] for c in range(NCORES)], axis=0)
    return np.ascontiguousarray(out_t.T).reshape(B, S, D)


LAST_RESULT = {}
